# revision 30
# baseline (speedup 1.0000x reference)
"""Causal self-attention on 8 TRN2 NeuronCores (Bass/Tile, SPMD).

Problem: B=4, T=2048, C=1024, H=16, D=64, fp32 in/out.

Sharding: core i = (batch b=i//2, parity p=i%2). Each core computes ALL 16
heads for 8 of the 16 128-wide query tiles of its batch: parity 0 owns
global q-tiles {0,3,4,7,8,11,12,15}, parity 1 owns {1,2,5,6,9,10,13,14}.
Sorted by causal extent, both parities pad to the SAME per-slot key-tile
extents E = [2,4,6,8,10,12,14,16] -> every core runs the IDENTICAL
instruction stream (SPMD); causality handled by host-supplied mask data
(mask m is triangular/ones/zeros depending on parity). K/V are computed
for the full sequence on both cores of a batch.

All tensor data is bf16 (PSUM accumulation fp32). Schedule is built to
keep the PE continuously busy (it only reaches its 2.4 GHz p-state after
~3us of uninterrupted work):
  part 1: K^T/V for t-superslab 0 + all of Q^T (j-major matmuls, weights
          stationary; PSUM evacuated with +bias on the Act engine).
  X sweep: attention pass 0 (slots 0-3, key tiles < 8) for all j,
          software-pipelined (PV lags S/exp by 2 pairs), with superslab-1
          K^T/V matmuls interleaved as filler so exp latency never idles
          the PE.
  Y sweep: attention pass 1 (slots 4-7, all 16 key tiles), two j's
          interleaved (independent chains) as mutual filler.
  tail:   output projection Y = O^T.T @ Wproj + bias_eff from SBUF
          (bias_eff = bproj + bv @ Wproj, exact since softmax rows sum
          to 1).

Attention inner scheme: q-tiles needing key-tile m form a contiguous
suffix of the slot-sorted Q columns, so S^T(m) = K_m^T Q_suffix is ONE
long matmul (N=512..128) per (half, m); matmul PSUM outputs are split at
512-col PSUM bank boundaries (HW corrupts writes that cross a bank);
batched exp on ScalarE; per-pair diagonal masks on DVE; PV accumulates
into per-pass [65,512] PSUM accumulators (row 64 = softmax denominator);
normalize via DVE reciprocal + GpSimd partition broadcast; O^T in SBUF.

Host: slices/transposes/casts inputs to bf16, precomputes masks +
effective bias, reassembles the 8 per-core [1024,1024] outputs.
"""
import os
import numpy as np
import ml_dtypes

import concourse.bacc as bacc
import concourse.mybir as mybir
import concourse.tile as tile
from concourse.bass_utils import run_bass_kernel_spmd

B, T, C, H, D = 4, 2048, 1024, 16, 64
F32 = mybir.dt.float32
BF16 = mybir.dt.bfloat16
NPBF = ml_dtypes.bfloat16
VA_W = H * (D + 1)            # 1040: V_aug cols = 16 heads x (64 + ones)
OWNT = [[0, 3, 4, 7, 8, 11, 12, 15], [1, 2, 5, 6, 9, 10, 13, 14]]
EXT = [2, 4, 6, 8, 10, 12, 14, 16]   # padded key-tile extent per slot

_cache = {}


def _build():
    nc = bacc.Bacc("TRN2", target_bir_lowering=False, debug=False,
                   enable_asserts=False, num_devices=8)
    def din(name, shape, dt=BF16):
        return nc.dram_tensor(name, list(shape), dt, kind="ExternalInput").ap()

    xt_d = din("xt", (C, T))            # x[b].T
    xq_d = din("xq", (C, 1024))         # own q columns of xt, slot-sorted
    wq_d = din("wq", (C, C))            # pre-scaled by 1/8
    wk_d = din("wk", (C, C))
    wv_d = din("wv", (C, C))
    wp_d = din("wp", (C, C))
    bq_d = din("bq", (8, 128, 1), F32)  # pre-scaled by 1/8
    bk_d = din("bk", (8, 128, 1), F32)
    bpeb_d = din("bpeb", (128, C), F32)
    mk_d = din("masks", (128, 16 * 128))
    y_d = nc.dram_tensor("y", [1024, C], F32, kind="ExternalOutput").ap()

    bypass = mybir.AluOpType.bypass
    mult = mybir.AluOpType.mult
    add = mybir.AluOpType.add
    EXP = mybir.ActivationFunctionType.Exp

    with tile.TileContext(nc) as tc:
        # ---------------- persistent tiles ------------------------------
        per = tc.alloc_tile_pool(name="per", bufs=1)
        KT = [per.tile([128, T], BF16, name=f"kt{j}", tag=f"kt{j}") for j in range(8)]
        VA = [per.tile([128, VA_W], BF16, name=f"va{m}", tag=f"va{m}") for m in range(16)]
        QT = [per.tile([128, 1024], BF16, name=f"qt{j}", tag=f"qt{j}") for j in range(8)]
        OT = [per.tile([128, 1024], BF16, name=f"ot{j}", tag=f"ot{j}") for j in range(8)]
        MK = per.tile([128, 16 * 128], BF16, name="mk", tag="mk")
        bks = [per.tile([128, 1], F32, name=f"bks{j}", tag=f"bks{j}") for j in range(8)]
        bqs = [per.tile([128, 1], F32, name=f"bqs{j}", tag=f"bqs{j}") for j in range(8)]
        bpeb = per.tile([128, C], F32, name="bpeb", tag="bpeb")
        ones16 = per.tile([128, H], BF16, name="ones16", tag="ones16")

        wkvp = tc.alloc_tile_pool(name="wkvp", bufs=1)
        wkc = [wkvp.tile([128, C], BF16, name=f"wkc{c}", tag=f"wkc{c}") for c in range(8)]
        wvc = [wkvp.tile([128, C], BF16, name=f"wvc{c}", tag=f"wvc{c}") for c in range(8)]
        wpp = tc.alloc_tile_pool(name="wpp", bufs=1)
        wpc = [wpp.tile([128, C], BF16, name=f"wpc{c}", tag=f"wpc{c}") for c in range(8)]
        xs1p = tc.alloc_tile_pool(name="xs1p", bufs=1)
        xts1 = [xs1p.tile([128, 1024], BF16, name=f"x1_{c}", tag=f"x1_{c}")
                for c in range(8)]

        def k_block(ps1, xts, j, ss):
            """K^T block j for one t-superslab (16 matmuls + Act evac)."""
            pk = ps1.tile([128, 1024], F32, name="pk", tag="pk")
            for c in range(8):
                for sub in range(2):
                    nc.tensor.matmul(out=pk[:, 512*sub:512*(sub+1)],
                                     lhsT=wkc[c][:, 128*j:128*(j+1)],
                                     rhs=xts[c][:, 512*sub:512*(sub+1)],
                                     start=(c == 0), stop=(c == 7))
            nc.scalar.add(out=KT[j][:, 1024*ss:1024*(ss+1)], in_=pk[:],
                          add=bks[j][:])

        def v_block(ps1, xts, tt, ss):
            """V_aug t-tile m=8*ss+tt (16 matmuls + Act evac, rearranged)."""
            pv = ps1.tile([128, 1024], F32, name="pv", tag="pk")
            for c in range(8):
                for jc in range(2):
                    nc.tensor.matmul(out=pv[:, 512*jc:512*(jc+1)],
                                     lhsT=xts[c][:, 128*tt:128*(tt+1)],
                                     rhs=wvc[c][:, 512*jc:512*(jc+1)],
                                     start=(c == 0), stop=(c == 7))
            dst = VA[8*ss+tt][:].rearrange("p (h d) -> p h d", d=D+1)[:, :, 0:D]
            src = pv[:].rearrange("p (h d) -> p h d", d=D)
            nc.scalar.copy(out=dst, in_=src)

        # -------- part 1: superslab-0 K^T and V, all of Q^T -------------
        with tc.tile_pool(name="wqp", bufs=1) as wqp, \
             tc.tile_pool(name="xs0p", bufs=1) as xs0p, \
             tc.tile_pool(name="ps1", bufs=3, space="PSUM") as ps1:
            wqc = [wqp.tile([128, C], BF16, name=f"wqc{c}", tag=f"wqc{c}")
                   for c in range(8)]
            # priority DMAs: interleave K weights with x superslab 0
            xts0 = []
            for c in range(8):
                nc.sync.dma_start(out=wkc[c][:], in_=wk_d[128*c:128*(c+1), :])
                xt_t = xs0p.tile([128, 1024], BF16, name=f"x0_{c}", tag=f"x0_{c}")
                nc.sync.dma_start(out=xt_t[:], in_=xt_d[128*c:128*(c+1), 0:1024])
                xts0.append(xt_t)
            for j in range(8):
                nc.sync.dma_start(out=bks[j][:], in_=bk_d[j])
            for c in range(8):
                nc.sync.dma_start(out=wvc[c][:], in_=wv_d[128*c:128*(c+1), :])
            for c in range(8):
                nc.sync.dma_start(out=wqc[c][:], in_=wq_d[128*c:128*(c+1), :])
            for j in range(8):
                nc.sync.dma_start(out=bqs[j][:], in_=bq_d[j])
            nc.sync.dma_start(out=MK[:], in_=mk_d[:])
            nc.sync.dma_start(out=bpeb[:], in_=bpeb_d[:])
            for c in range(8):
                nc.sync.dma_start(out=xts1[c][:], in_=xt_d[128*c:128*(c+1), 1024:2048])
            # xq reuses the xts0 buffers (second tile per tag)
            xqc = []
            for c in range(8):
                xq_t = xs0p.tile([128, 1024], BF16, name=f"x0_{c}", tag=f"x0_{c}")
                nc.sync.dma_start(out=xq_t[:], in_=xq_d[128*c:128*(c+1), :])
                xqc.append(xq_t)

            nc.vector.memset(ones16[:], 1.0)
            ones16_3d = ones16[:].unsqueeze(2)
            for m in range(16):
                dst1 = VA[m][:].rearrange("p (h d) -> p h d", d=D+1)[:, :, D:D+1]
                nc.vector.tensor_copy(out=dst1, in_=ones16_3d)

            for j in range(8):
                k_block(ps1, xts0, j, 0)
            for tt in range(8):
                v_block(ps1, xts0, tt, 0)
            for j in range(8):
                pq = ps1.tile([128, 1024], F32, name="pq", tag="pk")
                for c in range(8):
                    for sub in range(2):
                        nc.tensor.matmul(out=pq[:, 512*sub:512*(sub+1)],
                                         lhsT=wqc[c][:, 128*j:128*(j+1)],
                                         rhs=xqc[c][:, 512*sub:512*(sub+1)],
                                         start=(c == 0), stop=(c == 7))
                nc.scalar.add(out=QT[j][:, :], in_=pq[:], add=bqs[j][:])

        # ---------------- attention helpers -----------------------------
        def emit_pair_packed(ps, att, ss_bufs, j, p, g):
            m0, m1 = 2*g, 2*g + 1
            so = max(0, g - 4*p)
            qc0 = 512*p + 128*so
            N = 512 - 128*so
            masked = (g >= 4*p)
            halves = [(0, [(0, m0), (1, m1)]), (1, [(2, m0), (3, m1)])]
            ss_t = ps.tile([128, 1024], F32, name="ss", tag="ss", bufs=ss_bufs)
            # adjacent quadrant pairs (h0/h64) stream concurrently; their
            # dsts land in different PSUM banks (u0/u2, u1/u3)
            for ui in range(2):
                for half in range(2):
                    r0, r1 = 64*half, 64*(half+1)
                    u = 2*half + ui
                    m = (m0, m1)[ui]
                    nc.tensor.matmul(out=ss_t[:, N*u:N*(u+1)],
                                     lhsT=KT[j][r0:r1, 128*m:128*(m+1)],
                                     rhs=QT[j][r0:r1, qc0:qc0+N],
                                     tile_position=(r0, 0),
                                     start=True, stop=True)
            pt = att.tile([128, 1024], BF16, name="pt", tag="pt", bufs=12)
            nc.scalar.activation(out=pt[:, 0:4*N], in_=ss_t[:, 0:4*N], func=EXP)
            out = []
            for half, ums in halves:
                if masked:
                    for u, m in ums:
                        nc.vector.scalar_tensor_tensor(
                            out=pt[:, N*u:N*u+128], in0=pt[:, N*u:N*u+128],
                            scalar=0.0, in1=MK[:, 128*m:128*(m+1)],
                            op0=bypass, op1=mult)
                out.append((pt, N, so, ums, half))
            return out

        def emit_pair_unpacked(ps, att, ss_bufs, j, p, g):
            m0, m1 = 2*g, 2*g + 1
            so = max(0, g - 4*p)
            qc0 = 512*p + 128*so
            N = 512 - 128*so
            masked = (g >= 4*p)
            sst = [ps.tile([128, 1024], F32, name="ss", tag="ss", bufs=ss_bufs)
                   for _ in range(2)]
            # u-major, half-inner: adjacent quadrant pairs (h0/h64) stream
            # concurrently into different ss tiles (different banks)
            for u, m in ((0, m0), (1, m1)):
                for half in range(2):
                    r0, r1 = 64*half, 64*(half+1)
                    c0 = N * u           # split dst at PSUM bank boundaries
                    while c0 < N * (u + 1):
                        c1 = min(N * (u + 1), (c0 // 512 + 1) * 512)
                        nc.tensor.matmul(out=sst[half][:, c0:c1],
                                         lhsT=KT[j][r0:r1, 128*m:128*(m+1)],
                                         rhs=QT[j][r0:r1, qc0 + c0 - N*u:
                                                          qc0 + c1 - N*u],
                                         tile_position=(r0, 0),
                                         start=True, stop=True)
                        c0 = c1
            out = []
            for half in range(2):
                pt = att.tile([128, 1024], BF16, name="pt", tag="pt", bufs=12)
                nc.scalar.activation(out=pt[:, 0:2*N], in_=sst[half][:, 0:2*N],
                                     func=EXP)
                if masked:
                    for u, m in ((0, m0), (1, m1)):
                        nc.vector.scalar_tensor_tensor(
                            out=pt[:, N*u:N*u+128], in0=pt[:, N*u:N*u+128],
                            scalar=0.0, in1=MK[:, 128*m:128*(m+1)],
                            op0=bypass, op1=mult)
                out.append((pt, N, so, [(0, m0), (1, m1)], half))
            return out

        def emit_attn_pair(ps, att, ss_bufs, j, p, g):
            so = max(0, g - 4*p)
            N = 512 - 128*so
            if 4*N == 1024:     # packed halves land in separate PSUM banks
                return emit_pair_packed(ps, att, ss_bufs, j, p, g)
            return emit_pair_unpacked(ps, att, ss_bufs, j, p, g)

        def emit_pv(acc, j, mlast, items):
            for (ppt, pN, pso, ums, phalf) in items:
                hh = 2*j + phalf
                for u, m in ums:
                    nc.tensor.matmul(out=acc[phalf][:, 128*pso:512],
                                     lhsT=VA[m][:, 65*hh:65*(hh+1)],
                                     rhs=ppt[:, pN*u:pN*(u+1)],
                                     start=(m == 0), stop=(m == mlast),
                                     skip_group_check=True)

        def emit_norm(att, acc, j, p):
            for half in range(2):
                lsb = att.tile([1, 512], F32, name="lsb", tag=f"lsb{half}", bufs=1)
                nc.vector.tensor_copy(out=lsb[:], in_=acc[half][64:65, :])
                rsb = att.tile([1, 512], F32, name="rsb", tag=f"rsb{half}", bufs=1)
                nc.vector.reciprocal_approx_fast(rsb[:], lsb[:])
                rbb = att.tile([64, 512], F32, name="rbb", tag=f"rbb{half}", bufs=1)
                nc.gpsimd.partition_broadcast(rbb[:], rsb[:])
                nc.vector.scalar_tensor_tensor(
                    out=OT[j][64*half:64*(half+1), 512*p:512*(p+1)],
                    in0=acc[half][0:64, :], scalar=0.0, in1=rbb[:],
                    op0=bypass, op1=mult)

        LAG = 2
        # ---- X sweep: pass 0 for all j; superslab-1 K/V as PE filler ----
        with tc.tile_pool(name="attx", bufs=1) as attx, \
             tc.tile_pool(name="psx", bufs=1, space="PSUM") as psx:
            for c in range(8):
                nc.sync.dma_start(out=wpc[c][:], in_=wp_d[128*c:128*(c+1), :])
            def filler_steps():
                # superslab-1 K/V in half-block steps (8 matmuls per step)
                for j in range(8):
                    pk = psx.tile([128, 1024], F32, name="pk", tag="pk")
                    for c in range(8):
                        for sub in range(2):
                            nc.tensor.matmul(out=pk[:, 512*sub:512*(sub+1)],
                                             lhsT=wkc[c][:, 128*j:128*(j+1)],
                                             rhs=xts1[c][:, 512*sub:512*(sub+1)],
                                             start=(c == 0), stop=(c == 7))
                        if c == 3:
                            yield
                    nc.scalar.add(out=KT[j][:, 1024:2048], in_=pk[:], add=bks[j][:])
                    yield
                for tt in range(8):
                    pv = psx.tile([128, 1024], F32, name="pv", tag="pk")
                    for c in range(8):
                        for jc in range(2):
                            nc.tensor.matmul(out=pv[:, 512*jc:512*(jc+1)],
                                             lhsT=xts1[c][:, 128*tt:128*(tt+1)],
                                             rhs=wvc[c][:, 512*jc:512*(jc+1)],
                                             start=(c == 0), stop=(c == 7))
                        if c == 3:
                            yield
                    dst = VA[8+tt][:].rearrange("p (h d) -> p h d", d=D+1)[:, :, 0:D]
                    src = pv[:].rearrange("p (h d) -> p h d", d=D)
                    nc.scalar.copy(out=dst, in_=src)
                    yield
            fgen = filler_steps()
            pend = []        # carries the PV pipeline ACROSS j boundaries

            def pump_x(drain=False):
                while pend and (drain or len(pend) > LAG):
                    pacc, pj, pg, items = pend.pop(0)
                    emit_pv(pacc, pj, 7, items)
                    if pg == 3:
                        emit_norm(attx, pacc, pj, 0)
            for j in range(8):
                acc = [psx.tile([65, 512], F32, name=f"acc{h}", tag=f"acc{h}")
                       for h in range(2)]
                for g in range(4):
                    pend.append((acc, j, g, emit_attn_pair(psx, attx, 2, j, 0, g)))
                    next(fgen, None)
                    pump_x()
            pump_x(drain=True)
            for _ in fgen:
                pass
        xs1p.release()

        # ---- Y sweep: pass 1, two j's interleaved; proj tail reuses the
        # ss-tag PSUM buffers (no pool boundary, PE never idles) ---------
        with tc.tile_pool(name="atty", bufs=1) as atty, \
             tc.tile_pool(name="psy", bufs=1, space="PSUM") as psy:
            pend = []        # PV pipeline carried across jp boundaries

            def pump_y(drain=False):
                while pend and (drain or len(pend) > 4):
                    pacc, pj, pg, items = pend.pop(0)
                    emit_pv(pacc, pj, 15, items)
                    if pg == 7:
                        emit_norm(atty, pacc, pj, 1)
            for jp in range(4):
                js = (2*jp, 2*jp + 1)
                acc = {jj: [psy.tile([65, 512], F32, name=f"acc{jj%2}{h}",
                                     tag=f"acc{jj%2}{h}") for h in range(2)]
                       for jj in js}
                for g in range(8):
                    for jj in js:
                        pend.append((acc[jj], jj, g,
                                     emit_attn_pair(psy, atty, 2, jj, 1, g)))
                        pump_y()
            pump_y(drain=True)
            for ti in range(8):
                tp = ti % 2      # alternate freed acc banks -> 2-deep pipeline
                py = [psy.tile([128, 512], F32, name=f"py{jc}",
                               tag=f"acc{tp}{jc}") for jc in range(2)]
                for ci in range(8):
                    for jc in range(2):
                        nc.tensor.matmul(out=py[jc][:],
                                         lhsT=OT[ci][:, 128*ti:128*(ti+1)],
                                         rhs=wpc[ci][:, 512*jc:512*(jc+1)],
                                         start=(ci == 0), stop=(ci == 7))
                for jc in range(2):
                    ysb = atty.tile([128, 512], F32, name="ysb", tag="ysb",
                                    bufs=4)
                    nc.vector.scalar_tensor_tensor(
                        out=ysb[:], in0=py[jc][:], scalar=0.0,
                        in1=bpeb[:, 512*jc:512*(jc+1)],
                        op0=bypass, op1=add)
                    nc.sync.dma_start(
                        out=y_d[128*ti:128*(ti+1), 512*jc:512*(jc+1)],
                        in_=ysb[:])
        wpp.release()
        wkvp.release()
        per.release()

    nc.compile()
    return nc


def _get_nc():
    if "nc" not in _cache:
        _cache["nc"] = _build()
    return _cache["nc"]


def _host_prep(x, Wqkv, bqkv, Wproj, bproj):
    x = np.ascontiguousarray(np.asarray(x, dtype=np.float32))
    Wqkv = np.asarray(Wqkv, dtype=np.float32)
    bqkv = np.asarray(bqkv, dtype=np.float32)
    Wproj = np.ascontiguousarray(np.asarray(Wproj, dtype=np.float32))
    bproj = np.asarray(bproj, dtype=np.float32)

    wq = np.ascontiguousarray(Wqkv[:, :C] * np.float32(0.125)).astype(NPBF)
    wk = np.ascontiguousarray(Wqkv[:, C:2*C]).astype(NPBF)
    wv = np.ascontiguousarray(Wqkv[:, 2*C:]).astype(NPBF)
    wp = Wproj.astype(NPBF)
    bq8 = (bqkv[:C] * np.float32(0.125)).reshape(8, 128, 1).copy()
    bk8 = bqkv[C:2*C].reshape(8, 128, 1).copy()
    bv = bqkv[2*C:]
    bpe = (bproj.astype(np.float64) + bv.astype(np.float64) @ Wproj.astype(np.float64)).astype(np.float32)
    bpeb = np.ascontiguousarray(np.broadcast_to(bpe, (128, C)))

    ridx = np.arange(128)[:, None]
    cidx = np.arange(128)[None, :]
    tri = (ridx <= cidx)
    masks = []
    for par in range(2):
        mk = np.zeros((128, 16 * 128), dtype=NPBF)
        for m in range(16):
            g = OWNT[par][m // 2]
            if m < g:
                mk[:, 128*m:128*(m+1)] = 1
            elif m == g:
                mk[:, 128*m:128*(m+1)] = tri
        masks.append(mk)

    in_maps = []
    for core in range(8):
        b, par = core // 2, core % 2
        xt = np.ascontiguousarray(x[b].T.astype(NPBF))
        xq = np.ascontiguousarray(
            np.concatenate([xt[:, 128*g:128*(g+1)] for g in OWNT[par]], axis=1))
        in_maps.append(dict(xt=xt, xq=xq, wq=wq, wk=wk, wv=wv, wp=wp,
                            bq=bq8, bk=bk8, bpeb=bpeb, masks=masks[par]))
    return in_maps


def kernel(x, Wqkv, bqkv, Wproj, bproj):
    nc = _get_nc()
    in_maps = _host_prep(x, Wqkv, bqkv, Wproj, bproj)
    trace = bool(os.environ.get("BASS_TRACE"))
    if trace:
        try:
            import antenv.axon_hooks  # noqa: F401  (NTFF hook registry)
        except ImportError:
            trace = False
    res = run_bass_kernel_spmd(nc, in_maps, list(range(8)), trace=trace)
    _cache["last_exec_time_ns"] = res.exec_time_ns
    _cache["last_res"] = res
    out = np.empty((B, T, C), dtype=np.float32)
    for core in range(8):
        b, par = core // 2, core % 2
        y = res.results[core]["y"]
        for si, g in enumerate(OWNT[par]):
            out[b, 128*g:128*(g+1)] = y[128*si:128*(si+1)]
    return out
